# revision 1
# baseline (speedup 1.0000x reference)
"""Trainium2 Bass kernel for nn_ExportGatedDeltaNet (gated linear attention
with depthwise conv, chunked recurrence).

Self-contained: hardcodes shapes/sharding. Sharding: 8-way tensor-parallel
over heads (each core owns 4 of the 32 value heads / 2 of the 16 key heads);
both batch elements are processed sequentially on every core. Each core
computes a full [B, T, C] partial of the output projection over its head
slice; the host sums the 8 partials.
"""

import numpy as np
import ml_dtypes

import concourse.bass as bass
import concourse.tile as tile
from concourse import mybir
from concourse.vector_clock import ScopedClock, VectorClock
from concourse.bass_utils import run_bass_kernel_spmd

F32 = mybir.dt.float32
F32R = mybir.dt.float32r
BF16 = mybir.dt.bfloat16
AF = mybir.ActivationFunctionType
OP = mybir.AluOpType
BF16_NP = ml_dtypes.bfloat16

NK, NV, DK, DV, KCONV, C = 16, 32, 128, 128, 4, 2048
KEY = NK * DK            # 2048
B, T = 2, 2048
L = 128                  # recurrence chunk length
TB = 512                 # t-block
NTB = T // TB            # 4
NCH = TB // L            # chunks per t-block
NCORES = 8
EPS = 1e-6

# per-core head slice
VH = NV // NCORES        # 4 value heads
KH = NK // NCORES        # 2 key heads
QCH = KH * DK            # 256
VCH = VH * DV            # 512
ZCH = VH * DV            # 512
CONVCH = 2 * QCH + VCH   # 1024 channels through the conv
TOTCH = CONVCH + ZCH + 32 + VH  # 1572: ..., b(4), pad(28), a(4)
N_CT = C // 128          # 16 contraction tiles


def _walrus_safe_drain(self, tick_clock, wait_clock):
    # The container's walrus rejects >1 sync-wait on CTRL-class instructions;
    # split the final drain's waits across single-wait nops.
    vals = eval(repr(tick_clock.global_clock).replace("VectorClock", ""))
    for j, v in enumerate(vals):
        if not v:
            continue
        masked = [0] * len(vals)
        masked[j] = v
        nop_inst = self.nc.sync.nop(nofuse=True)
        wait_clock.add_sem_waits(
            nop_inst.ins, ScopedClock({None: VectorClock(masked)})
        )
    self.nc.sync.drain()
    self.nc.all_engine_barrier()
    popped = self.nc._tile_sem_poison_stack.pop()
    assert popped is self._sem_poison
    self.nc.clear_and_free_semaphores(list(self.sems.allocated().values()))
    self.nc.all_engine_barrier()


tile.TileContext._drain_and_barrier = _walrus_safe_drain


# The container's walrus rejects >1 sync-wait on any instruction. Tile's
# semaphore pass emits multi-wait instructions, so split them at the BIR-JSON
# level: hoist all but one wait onto NoOps (same engine) inserted just before.
_orig_to_json_bytes = bass.Bass.to_json_bytes
_WSPLIT = [0]


def _split_multi_waits(self, *args, **kwargs):
    import json
    raw = _orig_to_json_bytes(self, *args, **kwargs)
    m = json.loads(raw)
    changed = False
    for f in m["functions"]:
        for bb in f["blocks"]:
            out_insts = []
            for inst in bb["instructions"]:
                si = inst.get("sync_info")
                waits = (si or {}).get("on_wait") or []
                if len(waits) > 1:
                    changed = True
                    for w in waits[:-1]:
                        _WSPLIT[0] += 1
                        out_insts.append({
                            "debug": inst.get("debug"),
                            "engine": inst["engine"],
                            "ins": [], "outs": [],
                            "name": f"I-wsplit-{_WSPLIT[0]}",
                            "opcode": "NoOp",
                            "sync_info": {"on_update": [], "on_wait": [w]},
                        })
                    si["on_wait"] = [waits[-1]]
                out_insts.append(inst)
            bb["instructions"] = out_insts
    if not changed:
        return raw
    return json.dumps(m).encode()


bass.Bass.to_json_bytes = _split_multi_waits

# HWDGE DMAs execute on DMA-queue timelines, where a hoisted same-engine NoOp
# wait does not gate them. Route static DMAs through the SP sequencer instead
# so program order (and the NoOp wait splitting) applies to them too.
import concourse.bass_utils as _bu

_orig_run_command = _bu.run_command


def _patched_run_command(argv, **kwargs):
    argv = [a.replace("--assign-static-dmas-to-sp=false",
                      "--assign-static-dmas-to-sp=true") for a in argv]
    return _orig_run_command(argv, **kwargs)


_bu.run_command = _patched_run_command


def build_kernel():
    nc = bass.Bass(num_swdge_queues=4)

    xt = nc.dram_tensor("xt", [B, C, T], BF16, kind="ExternalInput")
    wt = nc.dram_tensor("wt", [C, TOTCH], BF16, kind="ExternalInput")
    wout = nc.dram_tensor("wout", [VCH, C], BF16, kind="ExternalInput")
    convw = nc.dram_tensor("convw", [128, CONVCH // 128, KCONV], F32,
                           kind="ExternalInput")
    halo = nc.dram_tensor("halo", [B, 128, CONVCH // 128, KCONV - 1], BF16,
                          kind="ExternalInput")
    s0 = nc.dram_tensor("s0", [B, VH, DK, DV], F32, kind="ExternalInput")
    dtb = nc.dram_tensor("dtb", [VH, 1], F32, kind="ExternalInput")
    nega = nc.dram_tensor("nega", [VH, 1], F32, kind="ExternalInput")
    normw = nc.dram_tensor("normw", [128, 1], F32, kind="ExternalInput")
    out = nc.dram_tensor("out", [B, T, C], F32, kind="ExternalOutput")

    # constants embedded in the NEFF
    ut_np = np.triu(np.ones((L, L), np.float32))              # [u,t]: u<=t
    UT = nc.inline_tensor(ut_np.astype(BF16_NP), name="UT")
    STA = nc.inline_tensor((1.0 - ut_np).astype(BF16_NP), name="STA")  # u>s
    ONESM = nc.inline_tensor(np.ones((L, L), BF16_NP), name="ONESM")
    NEGM = nc.inline_tensor(
        np.where(ut_np > 0, 0.0, -1e30).astype(np.float32), name="NEGM")
    IDENT = nc.inline_tensor(np.eye(8, dtype=np.float32), name="IDENT")
    IDENT128 = nc.inline_tensor(np.eye(128, dtype=BF16_NP), name="IDENT128")
    ONES_COL = nc.inline_tensor(np.ones((128, 1), BF16_NP), name="ONES_COL")
    ONES_ROW = nc.inline_tensor(np.ones((1, 128), np.float32), name="ONES_ROW")
    EPS_T = nc.inline_tensor(np.full((1, 1), EPS, np.float32), name="EPS_T")

    n_convt = CONVCH // 128   # 8 conv channel tiles
    n_zt = ZCH // 128         # 4
    n_wt = TOTCH // 128       # 12 full tiles + 8 extra cols handled separately

    from contextlib import ExitStack
    with nc.allow_low_precision(reason="bf16/f32r compute by design"), \
         tile.TileContext(nc) as tc, ExitStack() as stack:
        consts = stack.enter_context(tc.tile_pool(name="consts", bufs=1))
        wpool = stack.enter_context(tc.tile_pool(name="wpool", bufs=1))
        xpool = stack.enter_context(tc.tile_pool(name="xpool", bufs=2))
        rawp = stack.enter_context(tc.tile_pool(name="rawp", bufs=2))
        sbig = stack.enter_context(tc.tile_pool(name="sbig", bufs=2))
        stiny = stack.enter_context(tc.tile_pool(name="stiny", bufs=2))
        stiny3 = stack.enter_context(tc.tile_pool(name="stiny3", bufs=3))
        statep = stack.enter_context(tc.tile_pool(name="statep", bufs=1))
        sbig1 = stack.enter_context(tc.tile_pool(name="sbig1", bufs=1))
        pproj = stack.enter_context(tc.tile_pool(name="pproj", bufs=2, space="PSUM"))
        pddt = stack.enter_context(tc.tile_pool(name="pddt", bufs=2, space="PSUM"))
        pnorm = stack.enter_context(tc.tile_pool(name="pnorm", bufs=1, space="PSUM"))
        psmall = stack.enter_context(tc.tile_pool(name="psmall", bufs=3, space="PSUM"))

        # load constants to SBUF
        ut_sb = consts.tile([L, L], BF16)
        nc.gpsimd.dma_start(ut_sb[:], UT[:])
        sta_sb = consts.tile([L, L], BF16)
        nc.gpsimd.dma_start(sta_sb[:], STA[:])
        onesm_sb = consts.tile([L, L], BF16)
        nc.gpsimd.dma_start(onesm_sb[:], ONESM[:])
        negm_sb = consts.tile([L, L], F32)
        nc.gpsimd.dma_start(negm_sb[:], NEGM[:])
        ident_sb = consts.tile([8, 8], F32)
        nc.gpsimd.dma_start(ident_sb[:], IDENT[:])
        ident128_sb = consts.tile([128, 128], BF16)
        nc.gpsimd.dma_start(ident128_sb[:], IDENT128[:])
        onescol_sb = consts.tile([128, 1], BF16)
        nc.gpsimd.dma_start(onescol_sb[:], ONES_COL[:])
        onesrow_sb = consts.tile([1, 128], F32R)
        nc.gpsimd.dma_start(onesrow_sb[:], ONES_ROW[:].bitcast(F32R))
        eps_sb = consts.tile([1, 1], F32)
        nc.gpsimd.dma_start(eps_sb[:], EPS_T[:])
        convw_sb = consts.tile([128, n_convt, KCONV], F32)
        nc.gpsimd.dma_start(convw_sb[:], convw[:])
        dtb_sb = consts.tile([VH, 1], F32)
        nc.gpsimd.dma_start(dtb_sb[:], dtb[:])
        nega_sb = consts.tile([VH, 1], F32)
        nc.gpsimd.dma_start(nega_sb[:], nega[:])
        normw_sb = consts.tile([128, 1], F32)
        nc.gpsimd.dma_start(normw_sb[:], normw[:])

        # resident weights
        wt_sb = wpool.tile([128, N_CT, TOTCH], BF16)
        nc.gpsimd.dma_start(wt_sb[:], wt.rearrange("(ko p) f -> p ko f", p=128))
        wout_sb = wpool.tile([128, VH, C], BF16)
        nc.gpsimd.dma_start(wout_sb[:], wout.rearrange("(vo p) f -> p vo f", p=128))

        for b in range(B):
            S = statep.tile([128, VH, DV], F32R, tag="S")
            nc.gpsimd.dma_start(S[:], s0[b].rearrange("h d v -> d h v").bitcast(F32R))
            prev_raw = None
            for tb in range(NTB):
                tsl = slice(tb * TB, (tb + 1) * TB)
                xt_sb = xpool.tile([128, N_CT, TB], BF16, tag="xt")
                nc.gpsimd.dma_start(
                    xt_sb[:],
                    xt[b].rearrange("(ko p) t -> p ko t", p=128)[:, :, tsl])

                raw = rawp.tile([128, n_convt, TB + KCONV - 1], BF16, tag="raw")
                if tb == 0:
                    nc.gpsimd.dma_start(raw[:, :, 0:3], halo[b])
                else:
                    nc.any.tensor_copy(raw[:, :, 0:3], prev_raw[:, :, TB:TB + 3])
                prev_raw = raw

                z_sb = sbig.tile([128, n_zt, TB], BF16, tag="z")
                gsp = stiny.tile([VH, 2, TB], F32, tag="gsp")

                # ---- projections ----
                for cht in range(n_wt):
                    ps = pproj.tile([128, TB], F32, tag="proj")
                    for ct in range(N_CT):
                        nc.tensor.matmul(
                            ps[:], wt_sb[:, ct, cht * 128:(cht + 1) * 128],
                            xt_sb[:, ct, :],
                            start=(ct == 0), stop=(ct == N_CT - 1))
                    if cht < n_convt:
                        nc.any.tensor_copy(raw[:, cht, 3:TB + 3], ps[:])
                    else:
                        zi = cht - n_convt
                        nc.scalar.activation(z_sb[:, zi, :], ps[:], AF.Silu)
                # b/a projections (8 rows)
                ps_ba = pproj.tile([128, TB], F32, tag="proj")
                for ct in range(N_CT):
                    nc.tensor.matmul(
                        ps_ba[0:32 + VH, :], wt_sb[:, ct, n_wt * 128:TOTCH],
                        xt_sb[:, ct, :],
                        start=(ct == 0), stop=(ct == N_CT - 1))
                # psum rows 0:4 = b, rows 32:36 = a (32-aligned partition bases)
                # lnb = ln(sigmoid(b))   (walrus act tables can't mix softplus)
                nc.scalar.activation(gsp[:, 1, :], ps_ba[0:VH, :], AF.Sigmoid)
                nc.scalar.activation(gsp[:, 1, :], gsp[:, 1, :], AF.Ln)
                # softplus(a+dtb) = (a+dtb) - ln(sigmoid(a+dtb))
                lnt = stiny.tile([VH, TB], F32, tag="lnt")
                nc.scalar.activation(lnt[:], ps_ba[32:32 + VH, :],
                                     AF.Sigmoid, bias=dtb_sb[:])
                nc.scalar.activation(lnt[:], lnt[:], AF.Ln)
                nc.vector.scalar_tensor_tensor(
                    gsp[:, 0, :], ps_ba[32:32 + VH, :], dtb_sb[:], lnt[:],
                    OP.add, OP.subtract)
                nc.vector.tensor_scalar(gsp[:, 0, :], gsp[:, 0, :],
                                        nega_sb[:], None, OP.mult)

                # ---- conv + silu + q/k norm ----
                qn_sb = sbig.tile([128, KH, TB], BF16, tag="qn")
                kn_sb = sbig.tile([128, KH, TB], BF16, tag="kn")
                v_sb = sbig.tile([128, VH, TB], BF16, tag="v")
                for cht in range(n_convt):
                    acc = stiny.tile([128, TB], F32, tag="convacc")
                    nc.vector.tensor_scalar(
                        acc[:], raw[:, cht, 0:TB], convw_sb[:, cht, 0:1],
                        None, OP.mult)
                    for j in range(1, KCONV):
                        nc.vector.scalar_tensor_tensor(
                            acc[:], raw[:, cht, j:TB + j],
                            convw_sb[:, cht, j:j + 1], acc[:],
                            OP.mult, OP.add)
                    if cht < 2 * KH:  # q or k tile -> silu fp32 then normalize
                        f = stiny.tile([128, TB], F32, tag="qkf")
                        nc.scalar.activation(f[:], acc[:], AF.Silu)
                        sq = stiny.tile([128, TB], BF16, tag="sq")
                        nc.vector.tensor_tensor(sq[:], f[:], f[:], OP.mult)
                        ssq = pnorm.tile([1, TB], F32, tag="nrm")
                        for nn_ in range(TB // 512):
                            nc.tensor.matmul(
                                ssq[:, nn_ * 512:(nn_ + 1) * 512],
                                onescol_sb[:],
                                sq[:, nn_ * 512:(nn_ + 1) * 512],
                                start=True, stop=True)
                        rinv = stiny.tile([1, TB], F32R, tag="sroot")
                        nc.scalar.activation(rinv[:], ssq[:], AF.Sqrt)
                        nc.vector.tensor_scalar(rinv[:], rinv[:], 1e-12,
                                                None, OP.max)
                        nc.vector.reciprocal(rinv[:], rinv[:])
                        rb = pnorm.tile([128, TB], F32, tag="nrm")
                        for nn_ in range(TB // 512):
                            nc.tensor.matmul(
                                rb[:, nn_ * 512:(nn_ + 1) * 512],
                                onesrow_sb[:],
                                rinv[:, nn_ * 512:(nn_ + 1) * 512],
                                start=True, stop=True)
                        dst = qn_sb if cht < KH else kn_sb
                        nc.vector.tensor_tensor(dst[:, cht % KH, :], f[:],
                                                rb[:], OP.mult)
                    else:  # v tile
                        vi = cht - 2 * KH
                        nc.scalar.activation(v_sb[:, vi, :], acc[:], AF.Silu)

                og_sb = sbig1.tile([128, VH, TB], BF16, tag="og")

                # ---- chunks ----
                for c in range(NCH):
                    t0 = c * L
                    ktT = stiny3.tile([128, KH, L], BF16, tag="ktT")
                    for kh in range(KH):
                        nc.sync.dma_start_transpose(
                            ktT[:, kh, :], kn_sb[:, kh, t0:t0 + L])
                    vT = stiny3.tile([128, VH, L], BF16, tag="vT")
                    for h in range(VH):
                        nc.sync.dma_start_transpose(
                            vT[:, h, :], v_sb[:, h, t0:t0 + L])
                    # transpose gsp chunk -> gspT [128, 2VH]
                    tps = psmall.tile([128, L], F32, tag="mm128")
                    nc.tensor.transpose(tps[:, 0:VH],
                                        gsp[:, 0, t0:t0 + L], ident_sb[0:VH, 0:VH])
                    nc.tensor.transpose(tps[:, VH:2 * VH],
                                        gsp[:, 1, t0:t0 + L], ident_sb[0:VH, 0:VH])
                    gspT = stiny3.tile([128, 2 * VH], F32, tag="gspT")
                    nc.any.tensor_copy(gspT[:], tps[:, 0:2 * VH])

                    Ball = stiny.tile([128, VH, L], BF16, tag="Ball")
                    for h in range(VH):
                        nc.vector.tensor_scalar(
                            Ball[:, h, :], ut_sb[:], gspT[:, h:h + 1],
                            None, OP.mult)
                    Dps = pddt.tile([128, VH * L], F32, tag="ddt")
                    nc.tensor.matmul(Dps[:], sta_sb[:],
                                     Ball[:].rearrange("p a b -> p (a b)"),
                                     start=True, stop=True)
                    dtps = pddt.tile([128, VH * L], F32, tag="ddt")
                    nc.tensor.matmul(dtps[:], onesm_sb[:],
                                     Ball[:].rearrange("p a b -> p (a b)"),
                                     start=True, stop=True)
                    ebr = stiny.tile([128, VH, L], F32, tag="ebr")
                    nc.scalar.activation(
                        ebr[:].rearrange("p a b -> p (a b)"), dtps[:], AF.Exp)
                    Eall = stiny.tile([128, VH, L], F32, tag="Eall")
                    for h in range(VH):
                        nc.vector.scalar_tensor_tensor(
                            Eall[:, h, :], Dps[:, h * L:(h + 1) * L],
                            gspT[:, VH + h:VH + h + 1], negm_sb[:],
                            OP.add, OP.add)
                    Decay = Eall
                    nc.scalar.activation(
                        Decay[:].rearrange("p a b -> p (a b)"),
                        Eall[:].rearrange("p a b -> p (a b)"), AF.Exp)

                    Pps = []
                    for kh in range(KH):
                        pp = psmall.tile([128, L], F32, tag="mm128")
                        nc.tensor.matmul(pp[:], kn_sb[:, kh, t0:t0 + L],
                                         qn_sb[:, kh, t0:t0 + L],
                                         start=True, stop=True)
                        Pps.append(pp)

                    for h in range(VH):
                        kh = h // 2
                        qh = stiny3.tile([128, L], F32R, tag="qh")
                        nc.vector.tensor_tensor(
                            qh[:], qn_sb[:, kh, t0:t0 + L], ebr[:, h, :],
                            OP.mult)
                        PT = stiny3.tile([128, L], BF16, tag="PT")
                        nc.vector.tensor_tensor(PT[:], Pps[kh][:],
                                                Decay[:, h, :], OP.mult)
                        ops = psmall.tile([128, L], F32, tag="mm128")
                        nc.tensor.matmul(ops[:], S[:, h, :], qh[:],
                                         start=True, stop=False)
                        nc.tensor.matmul(ops[:], vT[:, h, :], PT[:],
                                         start=False, stop=True)
                        nc.vector.tensor_tensor(og_sb[:, h, t0:t0 + L],
                                                ops[:], z_sb[:, h, t0:t0 + L],
                                                OP.mult)
                        kt2 = stiny3.tile([128, L], BF16, tag="kt2")
                        nc.vector.tensor_scalar(
                            kt2[:], ktT[:, kh, :], Decay[:, h, L - 1:L],
                            None, OP.mult)
                        sps = psmall.tile([128, L], F32, tag="mm128")
                        nc.tensor.matmul(sps[:], kt2[:], vT[:, h, :],
                                         start=True, stop=True)
                        nc.vector.scalar_tensor_tensor(
                            S[:, h, :], S[:, h, :], ebr[:, h, L - 1:L],
                            sps[:], OP.mult, OP.add)

                # ---- gated rmsnorm + output projection ----
                ogn_sb = sbig.tile([128, VH, TB], BF16, tag="ogn")
                for h in range(VH):
                    sq2 = stiny.tile([128, TB], BF16, tag="sq")
                    nc.vector.tensor_tensor(sq2[:], og_sb[:, h, :],
                                            og_sb[:, h, :], OP.mult)
                    ssq2 = pnorm.tile([1, TB], F32, tag="nrm")
                    for nn_ in range(TB // 512):
                        nc.tensor.matmul(
                            ssq2[:, nn_ * 512:(nn_ + 1) * 512],
                            onescol_sb[:],
                            sq2[:, nn_ * 512:(nn_ + 1) * 512],
                            start=True, stop=True)
                    rinv2 = stiny.tile([1, TB], F32R, tag="sroot")
                    nc.scalar.activation(rinv2[:], ssq2[:], AF.Sqrt,
                                         bias=eps_sb[:], scale=1.0 / DV)
                    nc.vector.reciprocal(rinv2[:], rinv2[:])
                    rb2 = pnorm.tile([128, TB], F32, tag="nrm")
                    for nn_ in range(TB // 512):
                        nc.tensor.matmul(
                            rb2[:, nn_ * 512:(nn_ + 1) * 512],
                            onesrow_sb[:],
                            rinv2[:, nn_ * 512:(nn_ + 1) * 512],
                            start=True, stop=True)
                    nc.vector.scalar_tensor_tensor(
                        ogn_sb[:, h, :], og_sb[:, h, :], normw_sb[:],
                        rb2[:], OP.mult, OP.mult)

                for c in range(NCH):
                    rows = slice(tb * TB + c * L, tb * TB + (c + 1) * L)
                    for co in range(C // 512):
                        ops2 = pproj.tile([128, 512], F32, tag="proj")
                        for h in range(VH):
                            nc.tensor.matmul(
                                ops2[:],
                                ogn_sb[:, h, c * L:(c + 1) * L],
                                wout_sb[:, h, co * 512:(co + 1) * 512],
                                start=(h == 0), stop=(h == VH - 1))
                        ost = stiny.tile([128, 512], F32, tag="ost")
                        nc.any.tensor_copy(ost[:], ops2[:])
                        nc.gpsimd.dma_start(
                            out[b, rows, co * 512:(co + 1) * 512], ost[:])

    return nc


_NC_CACHE = None
LAST_RESULT = None


def kernel(**inputs):
    global _NC_CACHE, LAST_RESULT
    x = np.asarray(inputs["x"], np.float32)
    input_pos = np.asarray(inputs["input_pos"])
    W_qkv = np.asarray(inputs["W_qkv"], np.float32)
    W_z = np.asarray(inputs["W_z"], np.float32)
    W_b = np.asarray(inputs["W_b"], np.float32)
    W_a = np.asarray(inputs["W_a"], np.float32)
    conv_w = np.asarray(inputs["conv_w"], np.float32)[:, 0, :]
    dt_bias = np.asarray(inputs["dt_bias"], np.float32)
    A_log = np.asarray(inputs["A_log"], np.float32)
    norm_w = np.asarray(inputs["norm_w"], np.float32)
    W_out = np.asarray(inputs["W_out"], np.float32)
    conv_state = np.asarray(inputs["conv_state"], np.float32)
    rec_state = np.asarray(inputs["recurrent_state"], np.float32)

    keep = 0.0 if int(input_pos[0]) == 0 else 1.0
    conv_state = conv_state * keep
    rec_state = rec_state * keep

    xt_host = np.ascontiguousarray(x.transpose(0, 2, 1)).astype(BF16_NP)

    in_maps = []
    for core in range(NCORES):
        vh = slice(VH * core, VH * (core + 1))
        qrows = slice(QCH * core, QCH * (core + 1))
        krows = slice(KEY + QCH * core, KEY + QCH * (core + 1))
        vrows = slice(2 * KEY + VCH * core, 2 * KEY + VCH * (core + 1))
        zrows = slice(ZCH * core, ZCH * (core + 1))

        Wt = np.concatenate(
            [W_qkv[qrows], W_qkv[krows], W_qkv[vrows], W_z[zrows],
             W_b[vh], np.zeros((32 - VH, C), np.float32),
             W_a[vh]], axis=0)                    # [TOTCH, C]
        wt_host = np.ascontiguousarray(Wt.T).astype(BF16_NP)      # [C, TOTCH]
        wout_host = np.ascontiguousarray(
            W_out[:, VCH * core:VCH * (core + 1)].T).astype(BF16_NP)

        cw = np.concatenate([conv_w[qrows], conv_w[krows], conv_w[vrows]], 0)
        convw_host = np.ascontiguousarray(
            cw.reshape(CONVCH // 128, 128, KCONV).transpose(1, 0, 2))

        cs = np.concatenate([conv_state[:, qrows], conv_state[:, krows],
                             conv_state[:, vrows]], axis=1)       # [B,1024,4]
        halo_host = np.ascontiguousarray(
            cs[:, :, 1:4].reshape(B, CONVCH // 128, 128, 3)
            .transpose(0, 2, 1, 3)).astype(BF16_NP)

        s0_host = np.ascontiguousarray(rec_state[:, vh])          # [B,VH,DK,DV]
        dtb_host = np.ascontiguousarray(dt_bias[vh][:, None])
        nega_host = np.ascontiguousarray(-np.exp(A_log[vh])[:, None])
        normw_host = np.ascontiguousarray(norm_w[:, None])

        in_maps.append({
            "xt": xt_host, "wt": wt_host, "wout": wout_host,
            "convw": convw_host, "halo": halo_host, "s0": s0_host,
            "dtb": dtb_host, "nega": nega_host, "normw": normw_host,
        })

    if _NC_CACHE is None:
        _NC_CACHE = build_kernel()
    res = run_bass_kernel_spmd(_NC_CACHE, in_maps, core_ids=list(range(NCORES)))
    LAST_RESULT = res

    total = np.zeros((B, T, C), np.float32)
    for r in res.results:
        total += r["out"]
    return total



# revision 30
# speedup vs baseline: 1.6267x; 1.6267x over previous
"""Trainium2 Bass kernel for nn_ExportGatedDeltaNet (gated linear attention
with depthwise conv, chunked recurrence).

Self-contained: hardcodes shapes/sharding. Sharding: 8-way tensor-parallel
over heads (each core owns 4 of the 32 value heads / 2 of the 16 key heads);
both batch elements are processed sequentially on every core. Each core
computes a full [B, T, C] partial of the output projection over its head
slice; the host sums the 8 partials.

Software-pipelined: the qkv projection for t-block i+1 runs on the tensor
engine while the conv/gates/recurrence for block i run on vector/scalar/pool.
"""

import numpy as np
import ml_dtypes

import concourse.bass as bass
import concourse.tile as tile
from concourse import mybir
from concourse.vector_clock import ScopedClock, VectorClock
from concourse.bass_utils import run_bass_kernel_spmd

F32 = mybir.dt.float32
F32R = mybir.dt.float32r
BF16 = mybir.dt.bfloat16
AF = mybir.ActivationFunctionType
OP = mybir.AluOpType
BF16_NP = ml_dtypes.bfloat16

NK, NV, DK, DV, KCONV, C = 16, 32, 128, 128, 4, 2048
KEY = NK * DK            # 2048
B, T = 2, 2048
L = 128                  # recurrence chunk length
TB = 512                 # t-block
NTB = T // TB            # 4
NBLK = B * NTB           # 8 pipelined iterations
NCH = TB // L            # chunks per t-block
NCORES = 8
EPS = 1e-6

# per-core head slice
VH = NV // NCORES        # 4 value heads
KH = NK // NCORES        # 2 key heads
QCH = KH * DK            # 256
VCH = VH * DV            # 512
ZCH = VH * DV            # 512
CONVCH = 2 * QCH + VCH   # 1024 channels through the conv
TOTCH = CONVCH + ZCH + 32 + VH  # 1572: ..., b(4), pad(28), a(4)
N_CT = C // 128          # 16 contraction tiles
n_convt = CONVCH // 128  # 8 conv channel tiles
n_zt = ZCH // 128        # 4
n_wt = TOTCH // 128      # 12 full proj tiles + ba columns


def _walrus_safe_drain(self, tick_clock, wait_clock):
    # The container's walrus rejects >1 sync-wait on CTRL-class instructions;
    # split the final drain's waits across single-wait nops.
    vals = eval(repr(tick_clock.global_clock).replace("VectorClock", ""))
    for j, v in enumerate(vals):
        if not v:
            continue
        masked = [0] * len(vals)
        masked[j] = v
        nop_inst = self.nc.sync.nop(nofuse=True)
        wait_clock.add_sem_waits(
            nop_inst.ins, ScopedClock({None: VectorClock(masked)})
        )
    self.nc.sync.drain()
    self.nc.all_engine_barrier()
    popped = self.nc._tile_sem_poison_stack.pop()
    assert popped is self._sem_poison
    self.nc.clear_and_free_semaphores(list(self.sems.allocated().values()))
    self.nc.all_engine_barrier()


tile.TileContext._drain_and_barrier = _walrus_safe_drain


# The container's walrus rejects >1 sync-wait on any instruction. Tile's
# semaphore pass emits multi-wait instructions, so split them at the BIR-JSON
# level: hoist all but one wait onto NoOps (same engine) inserted just before.
_orig_to_json_bytes = bass.Bass.to_json_bytes
_WSPLIT = [0]


def _split_multi_waits(self, *args, **kwargs):
    import json
    raw = _orig_to_json_bytes(self, *args, **kwargs)
    m = json.loads(raw)
    changed = False
    for f in m["functions"]:
        for bb in f["blocks"]:
            out_insts = []
            for inst in bb["instructions"]:
                si = inst.get("sync_info")
                waits = (si or {}).get("on_wait") or []
                if len(waits) > 1:
                    changed = True
                    for w in waits[:-1]:
                        _WSPLIT[0] += 1
                        out_insts.append({
                            "debug": inst.get("debug"),
                            "engine": inst["engine"],
                            "ins": [], "outs": [],
                            "name": f"I-wsplit-{_WSPLIT[0]}",
                            "opcode": "NoOp",
                            "sync_info": {"on_update": [], "on_wait": [w]},
                        })
                    si["on_wait"] = [waits[-1]]
                out_insts.append(inst)
            bb["instructions"] = out_insts
    if not changed:
        return raw
    return json.dumps(m).encode()


bass.Bass.to_json_bytes = _split_multi_waits

# HWDGE DMAs execute on DMA-queue timelines, where a hoisted same-engine NoOp
# wait does not gate them. Route static DMAs through the SP sequencer instead
# so program order (and the NoOp wait splitting) applies to them too.
import concourse.bass_utils as _bu

_orig_run_command = _bu.run_command


def _patched_run_command(argv, **kwargs):
    argv = [a.replace("--assign-static-dmas-to-sp=false",
                      "--assign-static-dmas-to-sp=true") for a in argv]
    return _orig_run_command(argv, **kwargs)


_bu.run_command = _patched_run_command


def build_kernel():
    nc = bass.Bass(num_swdge_queues=4)

    xt = nc.dram_tensor("xt", [B, C, T], BF16, kind="ExternalInput")
    wt = nc.dram_tensor("wt", [C, TOTCH], BF16, kind="ExternalInput")
    wout = nc.dram_tensor("wout", [VCH, C], BF16, kind="ExternalInput")
    convw = nc.dram_tensor("convw", [128, n_convt, KCONV], F32,
                           kind="ExternalInput")
    halo = nc.dram_tensor("halo", [B, 128, n_convt, KCONV - 1], BF16,
                          kind="ExternalInput")
    s0 = nc.dram_tensor("s0", [B, DK, VH, DV], BF16, kind="ExternalInput")
    dtb = nc.dram_tensor("dtb", [VH, 1], F32, kind="ExternalInput")
    nega = nc.dram_tensor("nega", [VH, 1], F32, kind="ExternalInput")
    normw = nc.dram_tensor("normw", [128, 1], F32, kind="ExternalInput")
    out = nc.dram_tensor("out", [B, T, C], BF16, kind="ExternalOutput")

    # constants embedded in the NEFF
    ut_np = np.triu(np.ones((L, L), np.float32))              # [s,t]: s<=t
    UT = nc.inline_tensor(ut_np.astype(BF16_NP), name="UT")
    STA = nc.inline_tensor((1.0 - ut_np).astype(BF16_NP), name="STA")  # s>u
    stb_np = np.triu(np.ones((L, L), np.float32), 1)                   # s<u
    STB = nc.inline_tensor(stb_np.astype(BF16_NP), name="STB")
    ONESM = nc.inline_tensor(np.ones((L, L), BF16_NP), name="ONESM")
    negeye_np = np.zeros((L, VH, L), np.float32)
    for hh in range(VH):
        negeye_np[:, hh, :][np.eye(L, dtype=bool)] = -1e30
    NEGEYE = nc.inline_tensor(negeye_np.astype(BF16_NP), name="NEGEYE")
    IDENT4 = nc.inline_tensor(np.eye(VH, dtype=np.float32), name="IDENT4")
    IDENT128 = nc.inline_tensor(np.eye(128, dtype=BF16_NP), name="IDENT128")
    ONES_COL = nc.inline_tensor(np.ones((128, 1), BF16_NP), name="ONES_COL")
    ONES_ROW = nc.inline_tensor(np.ones((1, 128), BF16_NP), name="ONES_ROW")
    EPS_T = nc.inline_tensor(np.full((1, 1), EPS, np.float32), name="EPS_T")

    from contextlib import ExitStack
    with nc.allow_low_precision(reason="bf16 compute by design"), \
         tile.TileContext(nc) as tc, ExitStack() as stack:
        consts = stack.enter_context(tc.tile_pool(name="consts", bufs=1))
        wpool = stack.enter_context(tc.tile_pool(name="wpool", bufs=1))
        xpool = stack.enter_context(tc.tile_pool(name="xpool", bufs=2))
        rawp = stack.enter_context(tc.tile_pool(name="rawp", bufs=2))
        zp = stack.enter_context(tc.tile_pool(name="zp", bufs=2))
        sb1 = stack.enter_context(tc.tile_pool(name="sb1", bufs=1))
        sb2 = stack.enter_context(tc.tile_pool(name="sb2", bufs=2))
        statep = stack.enter_context(tc.tile_pool(name="statep", bufs=1))
        # PSUM pools: 8 banks of 2KB/partition, one bank per buffer:
        # p512 x2, pdd x2, pog x1, pnrm x1, paux x1, ptrb x1 -> 8 banks
        p512 = stack.enter_context(tc.tile_pool(name="p512", bufs=2, space="PSUM"))
        pdd = stack.enter_context(tc.tile_pool(name="pdd", bufs=2, space="PSUM"))
        pog = stack.enter_context(tc.tile_pool(name="pog", bufs=1, space="PSUM"))
        pnrm = stack.enter_context(tc.tile_pool(name="pnrm", bufs=1, space="PSUM"))
        paux = stack.enter_context(tc.tile_pool(name="paux", bufs=1, space="PSUM"))
        ptrb = stack.enter_context(tc.tile_pool(name="ptrb", bufs=1, space="PSUM"))

        # ---- load constants / resident weights ----
        ut_sb = consts.tile([L, L], BF16)
        nc.gpsimd.dma_start(ut_sb[:], UT[:])
        sta_sb = consts.tile([L, L], BF16)
        nc.gpsimd.dma_start(sta_sb[:], STA[:])
        stb_sb = consts.tile([L, L], BF16)
        nc.gpsimd.dma_start(stb_sb[:], STB[:])
        onesm_sb = consts.tile([L, L], BF16)
        nc.gpsimd.dma_start(onesm_sb[:], ONESM[:])
        negeye_sb = consts.tile([L, VH, L], BF16)
        nc.gpsimd.dma_start(negeye_sb[:], NEGEYE[:])
        ident4_sb = consts.tile([VH, VH], F32)
        nc.gpsimd.dma_start(ident4_sb[:], IDENT4[:])
        ident128_sb = consts.tile([128, 128], BF16)
        nc.gpsimd.dma_start(ident128_sb[:], IDENT128[:])
        onescol_sb = consts.tile([128, 1], BF16)
        nc.gpsimd.dma_start(onescol_sb[:], ONES_COL[:])
        onesrow_sb = consts.tile([1, 128], BF16)
        nc.gpsimd.dma_start(onesrow_sb[:], ONES_ROW[:])
        eps_sb = consts.tile([1, 1], F32)
        nc.gpsimd.dma_start(eps_sb[:], EPS_T[:])
        convw_sb = consts.tile([128, n_convt, KCONV], F32)
        nc.gpsimd.dma_start(convw_sb[:], convw[:])
        dtb_sb = consts.tile([VH, 1], F32)
        nc.gpsimd.dma_start(dtb_sb[:], dtb[:])
        nega_sb = consts.tile([VH, 1], F32)
        nc.gpsimd.dma_start(nega_sb[:], nega[:])
        normw_sb = consts.tile([128, 1], F32)
        nc.gpsimd.dma_start(normw_sb[:], normw[:])

        wt_sb = wpool.tile([128, N_CT, TOTCH], BF16)
        nc.gpsimd.dma_start(wt_sb[:], wt.rearrange("(ko p) f -> p ko f", p=128))
        wout_sb = wpool.tile([128, VH, C], BF16)
        nc.gpsimd.dma_start(wout_sb[:], wout.rearrange("(vo p) f -> p vo f", p=128))

        S = statep.tile([128, VH, DV], BF16, tag="S")
        nc.gpsimd.dma_start(S[:], s0[0])

        # per-iteration state handles, filled by stage P, consumed next iter
        st = [None] * NBLK   # dict per block

        def stage_P(i):
            """Projections for block i on tensor; raw/z copies on scalar;
            issues xt prefetch for block i+1."""
            b, tb = i // NTB, i % NTB
            tsl = slice(tb * TB, (tb + 1) * TB)
            d = {}
            st[i] = d
            if i == 0:
                xt_sb = xpool.tile([128, N_CT, TB], BF16, tag="xt")
                nc.gpsimd.dma_start(
                    xt_sb[:],
                    xt[0].rearrange("(ko p) t -> p ko t", p=128)[:, :, 0:TB])
                d["xt"] = xt_sb
            else:
                d["xt"] = st[i - 1].pop("xt_next")
            if i + 1 < NBLK:
                b2, tb2 = (i + 1) // NTB, (i + 1) % NTB
                t2 = slice(tb2 * TB, (tb2 + 1) * TB)
                xt2 = xpool.tile([128, N_CT, TB], BF16, tag="xt")
                nc.gpsimd.dma_start(
                    xt2[:],
                    xt[b2].rearrange("(ko p) t -> p ko t", p=128)[:, :, t2])
                d["xt_next"] = xt2
            xt_sb = d["xt"]

            raw = rawp.tile([128, n_convt, TB + KCONV - 1], BF16, tag="raw")
            d["raw"] = raw
            if tb == 0:
                nc.gpsimd.dma_start(raw[:, :, 0:3], halo[b])
            else:
                nc.scalar.copy(raw[:, :, 0:3],
                               st[i - 1]["raw"][:, :, TB:TB + 3])

            zr = zp.tile([128, n_zt, TB], BF16, tag="zr")
            d["z"] = zr
            gsp = zp.tile([VH, 2, TB], F32, tag="gsp")
            d["gsp"] = gsp

            for cht in range(n_wt):
                ps = p512.tile([128, TB], F32, tag="p512")
                for ct in range(N_CT):
                    nc.tensor.matmul(
                        ps[:], wt_sb[:, ct, cht * 128:(cht + 1) * 128],
                        xt_sb[:, ct, :],
                        start=(ct == 0), stop=(ct == N_CT - 1))
                if cht < n_convt:
                    nc.scalar.copy(raw[:, cht, 3:TB + 3], ps[:])
                else:
                    nc.scalar.copy(zr[:, cht - n_convt, :], ps[:])
            ps_ba = p512.tile([128, TB], F32, tag="p512")
            for ct in range(N_CT):
                nc.tensor.matmul(
                    ps_ba[0:32 + VH, :], wt_sb[:, ct, n_wt * 128:TOTCH],
                    xt_sb[:, ct, :],
                    start=(ct == 0), stop=(ct == N_CT - 1))
            d["ps_ba"] = ps_ba

        def stage_gates(i):
            """b/a gate math: exp/ln table only.
            gsp[:,0,:] = g = -exp(A_log)*softplus(a+dtb)
            gsp[:,1,:] = softplus(-b) = -ln(sigmoid(b))   (negated at use)"""
            d = st[i]
            gsp, ps_ba = d["gsp"], d.pop("ps_ba")
            nc.scalar.activation(gsp[:, 1, :], ps_ba[0:VH, :], AF.Exp,
                                 scale=-1.0)
            nc.scalar.activation(gsp[:, 1, :], gsp[:, 1, :], AF.Ln, bias=1.0)
            nc.vector.tensor_scalar(gsp[:, 1, :], gsp[:, 1, :], -1.0,
                                    None, OP.mult)
            nc.scalar.activation(gsp[:, 0, :], ps_ba[32:32 + VH, :], AF.Exp,
                                 bias=dtb_sb[:])
            nc.scalar.activation(gsp[:, 0, :], gsp[:, 0, :], AF.Ln, bias=1.0)
            nc.vector.tensor_scalar(gsp[:, 0, :], gsp[:, 0, :], nega_sb[:],
                                    None, OP.mult)

        def stage_conv(i):
            """Depthwise conv (vector, bf16)."""
            d = st[i]
            raw = d["raw"]
            accs = sb2.tile([128, n_convt, TB], BF16, tag="convacc")
            d["convacc"] = accs
            for cht in range(n_convt):
                acc = accs[:, cht, :]
                nc.vector.tensor_scalar(
                    acc, raw[:, cht, 0:TB], convw_sb[:, cht, 0:1],
                    None, OP.mult)
                for j in range(1, KCONV):
                    nc.vector.scalar_tensor_tensor(
                        acc, raw[:, cht, j:TB + j],
                        convw_sb[:, cht, j:j + 1], acc,
                        OP.mult, OP.add)

        def stage_silu(i):
            """All silus for block i (single table visit) + sq for qk norm."""
            d = st[i]
            accs = d.pop("convacc")
            f = sb1.tile([128, 2 * KH, TB], BF16, tag="f")
            v_sb = sb1.tile([128, VH, TB], BF16, tag="v")
            d["f"], d["v"] = f, v_sb
            for cht in range(n_convt):
                if cht < 2 * KH:
                    nc.scalar.activation(f[:, cht, :], accs[:, cht, :], AF.Silu)
                else:
                    nc.scalar.activation(v_sb[:, cht - 2 * KH, :],
                                         accs[:, cht, :], AF.Silu)
            z = d["z"]
            for zi in range(n_zt):
                nc.scalar.activation(z[:, zi, :], z[:, zi, :], AF.Silu)

        def stage_sq(i):
            d = st[i]
            f = d["f"]
            sq = sb1.tile([128, 2 * KH, TB], BF16, tag="sq")
            d["sq"] = sq
            nc.vector.tensor_tensor(
                sq[:].rearrange("p a b -> p (a b)"),
                f[:].rearrange("p a b -> p (a b)"),
                f[:].rearrange("p a b -> p (a b)"), OP.mult)

        def stage_qknorm(i):
            """q/k L2-normalize: ssq -> sqrt -> recip_fast -> bcast -> mult."""
            d = st[i]
            f, sq = d.pop("f"), d.pop("sq")
            qn = sb1.tile([128, KH, TB], BF16, tag="qn")
            kn = sb1.tile([128, KH, TB], BF16, tag="kn")
            d["qn"], d["kn"] = qn, kn
            for k in range(2 * KH):
                ssq = pnrm.tile([128, TB], F32, tag="nrm")
                nc.tensor.matmul(ssq[0:1, :], onescol_sb[:], sq[:, k, :],
                                 start=True, stop=True)
                # rinv = 1/sqrt(ssq) = exp(-0.5*ln(ssq)); ln/exp share a table
                rt = sb2.tile([1, TB], F32, tag="rinv")
                nc.scalar.activation(rt[:], ssq[0:1, :], AF.Ln)
                rt2 = sb2.tile([1, TB], BF16, tag="rinv2q")
                nc.scalar.activation(rt2[:], rt[:], AF.Exp, scale=-0.5)
                rb = pnrm.tile([128, TB], F32, tag="nrm")
                nc.tensor.matmul(rb[:], onesrow_sb[:], rt2[:],
                                 start=True, stop=True)
                dst = qn if k < KH else kn
                nc.vector.tensor_tensor(dst[:, k % KH, :], f[:, k, :], rb[:],
                                        OP.mult)

        def stage_chunks(i):
            """Recurrence chunks for block i. og left in SBUF for stage_out."""
            d = st[i]
            gsp, qn, kn, v_sb, z = d["gsp"], d["qn"], d["kn"], d["v"], d["z"]
            og = sb1.tile([128, VH, TB], BF16, tag="og")
            d["og"] = og
            for c in range(NCH):
                t0 = c * L
                ch = slice(t0, t0 + L)
                # aux psum bank: gspT cols 0:8, P tiles cols 128:384
                aux = paux.tile([128, 512], F32, tag="aux")
                # gate transposes: gspT[:,0:VH]=g^T, [:,VH:2VH]=lnb^T
                nc.tensor.transpose(aux[:, 0:VH], gsp[:, 0, ch], ident4_sb[:])
                nc.tensor.transpose(aux[:, VH:2 * VH], gsp[:, 1, ch],
                                    ident4_sb[:])
                gspT = sb2.tile([128, 2 * VH], F32, tag="gspT")
                nc.scalar.copy(gspT[:], aux[:, 0:2 * VH])

                # Ball[s,(h,t)] = (s<=t) * g_h[s]
                Ball = sb2.tile([128, VH, L], BF16, tag="Ball")
                for h in range(VH):
                    nc.vector.tensor_scalar(
                        Ball[:, h, :], ut_sb[:], gspT[:, h:h + 1],
                        None, OP.mult)
                Ball_f = Ball[:].rearrange("p a b -> p (a b)")

                # dtps[u,(h,t)] = G_t  (bcast over u);  Dps = G_t - G_u - mask
                dtps = pdd.tile([128, VH * L], F32, tag="dd")
                nc.tensor.matmul(dtps[:], onesm_sb[:], Ball_f,
                                 start=True, stop=True)
                Dps = pdd.tile([128, VH * L], F32, tag="dd")
                nc.tensor.matmul(Dps[:], sta_sb[:], Ball_f,
                                 start=True, stop=False)
                nc.tensor.matmul(Dps[:], stb_sb[:],
                                 negeye_sb[:].rearrange("p a b -> p (a b)"),
                                 start=False, stop=True)

                ebr = sb2.tile([128, VH, L], BF16, tag="ebr")
                nc.scalar.activation(ebr[:].rearrange("p a b -> p (a b)"),
                                     dtps[:], AF.Exp)
                Decay = sb2.tile([128, VH, L], F32, tag="Decay")
                for h in range(VH):
                    nc.scalar.activation(
                        Decay[:, h, :], Dps[:, h * L:(h + 1) * L], AF.Exp,
                        bias=gspT[:, VH + h:VH + h + 1])

                # k^T, v^T via tensor transposes; kt2 = k^T * Decay[:,L-1]
                trb = ptrb.tile([128, 8, 128], BF16, tag="trb")
                for kh in range(KH):
                    nc.tensor.transpose(trb[:, kh, :], kn[:, kh, ch],
                                        ident128_sb[:])
                kt2 = sb2.tile([128, VH, L], BF16, tag="kt2")
                for h in range(VH):
                    nc.scalar.activation(kt2[:, h, :], trb[:, h // 2, :],
                                         AF.Copy, scale=Decay[:, h, L - 1:L])
                vT = sb2.tile([128, VH, L], BF16, tag="vT")
                for h in range(VH):
                    nc.tensor.transpose(trb[:, 2 + h, :], v_sb[:, h, ch],
                                        ident128_sb[:])
                nc.scalar.copy(vT[:], trb[:, 2:2 + VH, :])

                # P[u,t] per key head
                for kh in range(KH):
                    nc.tensor.matmul(aux[:, 128 * (kh + 1):128 * (kh + 2)],
                                     kn[:, kh, ch], qn[:, kh, ch],
                                     start=True, stop=True)

                # qh = qn * ebr (heads share key head, broadcast-AP repeat)
                qh = sb2.tile([128, VH, L], BF16, tag="qh")
                qn_rep = (qn[:, :, ch].unsqueeze(2)
                          .broadcast_to((128, KH, VH // KH, L)))
                nc.vector.tensor_tensor(
                    qh[:].rearrange("p (a r) b -> p a r b", a=KH),
                    qn_rep,
                    ebr[:].rearrange("p (a r) b -> p a r b", a=KH), OP.mult)
                # PT = P * Decay (P broadcast over the 2 heads per key head)
                PT = sb2.tile([128, VH, L], BF16, tag="PT")
                P_rep = (aux[:, 128:384].rearrange("p (a b) -> p a b", a=KH)
                         .unsqueeze(2).broadcast_to((128, KH, VH // KH, L)))
                nc.vector.tensor_tensor(
                    PT[:].rearrange("p (a r) b -> p a r b", a=KH),
                    P_rep,
                    Decay[:].rearrange("p (a r) b -> p a r b", a=KH), OP.mult)

                # out = S^T qh + vT^T PT ; S update
                ogps = pog.tile([128, VH * L], F32, tag="og")
                for h in range(VH):
                    sl = slice(h * L, (h + 1) * L)
                    nc.tensor.matmul(ogps[:, sl], S[:, h, :], qh[:, h, :],
                                     start=True, stop=False)
                    nc.tensor.matmul(ogps[:, sl], vT[:, h, :], PT[:, h, :],
                                     start=False, stop=True)
                nc.vector.tensor_tensor(
                    og[:, :, ch], ogps[:].rearrange("p (a b) -> p a b", a=VH),
                    z[:, :, ch], OP.mult)
                sps = pog.tile([128, VH * L], F32, tag="og", name="sps")
                for h in range(VH):
                    nc.tensor.matmul(sps[:, h * L:(h + 1) * L], kt2[:, h, :],
                                     vT[:, h, :], start=True, stop=True)
                for h in range(VH):
                    nc.vector.scalar_tensor_tensor(
                        S[:, h, :], S[:, h, :], ebr[:, h, L - 1:L],
                        sps[:, h * L:(h + 1) * L], OP.mult, OP.add)

        def stage_out(i):
            """Gated RMSNorm + output projection + store, per chunk."""
            b, tb = i // NTB, i % NTB
            d = st[i]
            og = d.pop("og")
            for c in range(NCH):
                ch = slice(c * L, (c + 1) * L)
                sq2 = sb2.tile([128, VH, L], BF16, tag="sq2")
                nc.vector.tensor_tensor(sq2[:], og[:, :, ch], og[:, :, ch],
                                        OP.mult)
                ssq = pnrm.tile([128, VH * L], F32, tag="nrm")
                nc.tensor.matmul(ssq[0:1, :], onescol_sb[:],
                                 sq2[:].rearrange("p a b -> p (a b)"),
                                 start=True, stop=True)
                # 1/sqrt(mean+eps) = exp(-0.5*ln(ssq/DV + eps))
                rt = sb2.tile([1, VH * L], F32, tag="rinv2")
                nc.scalar.activation(rt[:], ssq[0:1, :], AF.Ln,
                                     bias=eps_sb[:], scale=1.0 / DV)
                rt2 = sb2.tile([1, VH * L], BF16, tag="rinv2o")
                nc.scalar.activation(rt2[:], rt[:], AF.Exp, scale=-0.5)
                rb = pnrm.tile([128, VH * L], F32, tag="nrm")
                nc.tensor.matmul(rb[:], onesrow_sb[:], rt2[:],
                                 start=True, stop=True)
                ogn = sb2.tile([128, VH, L], BF16, tag="ogn")
                nc.vector.scalar_tensor_tensor(
                    ogn[:], og[:, :, ch], normw_sb[:],
                    rb[:].rearrange("p (a b) -> p a b", a=VH),
                    OP.mult, OP.mult)

                rows = slice(tb * TB + c * L, tb * TB + (c + 1) * L)
                for co in range(C // 512):
                    ops2 = p512.tile([128, 512], F32, tag="p512")
                    for h in range(VH):
                        nc.tensor.matmul(
                            ops2[:], ogn[:, h, :],
                            wout_sb[:, h, co * 512:(co + 1) * 512],
                            start=(h == 0), stop=(h == VH - 1))
                    ost = sb2.tile([128, 512], BF16, tag="ost")
                    if co % 2 == 0:
                        nc.scalar.copy(ost[:], ops2[:])
                    else:
                        nc.vector.tensor_copy(ost[:], ops2[:])
                    nc.sync.dma_start(
                        out[b, rows, co * 512:(co + 1) * 512], ost[:])

        # ---------------- pipelined main loop ----------------
        for i in range(NBLK + 1):
            if i < NBLK:
                stage_P(i)
                if i == 0:
                    stage_gates(0)
            if i >= 1:
                j = i - 1
                stage_qknorm(j)
                if i < NBLK:
                    stage_gates(i)
                stage_chunks(j)
                stage_out(j)
            if i < NBLK:
                stage_conv(i)
                stage_silu(i)
                stage_sq(i)
            if i == NTB - 1:
                # recurrent state for b=1; emitted after chunks(NTB-2) so the
                # pool-queue WAR wait cannot deadlock (only chunks(NTB-1)
                # still reads S(b=0), and it is emitted after this point...
                pass
            if i == NTB:
                # after stage_chunks(NTB-1) consumed S(b=0)
                nc.gpsimd.dma_start(S[:], s0[1])

    return nc


_NC_CACHE = None
LAST_RESULT = None


def kernel(**inputs):
    global _NC_CACHE, LAST_RESULT
    x = np.asarray(inputs["x"], np.float32)
    input_pos = np.asarray(inputs["input_pos"])
    W_qkv = np.asarray(inputs["W_qkv"], np.float32)
    W_z = np.asarray(inputs["W_z"], np.float32)
    W_b = np.asarray(inputs["W_b"], np.float32)
    W_a = np.asarray(inputs["W_a"], np.float32)
    conv_w = np.asarray(inputs["conv_w"], np.float32)[:, 0, :]
    dt_bias = np.asarray(inputs["dt_bias"], np.float32)
    A_log = np.asarray(inputs["A_log"], np.float32)
    norm_w = np.asarray(inputs["norm_w"], np.float32)
    W_out = np.asarray(inputs["W_out"], np.float32)
    conv_state = np.asarray(inputs["conv_state"], np.float32)
    rec_state = np.asarray(inputs["recurrent_state"], np.float32)

    keep = 0.0 if int(input_pos[0]) == 0 else 1.0
    conv_state = conv_state * keep
    rec_state = rec_state * keep

    xt_host = np.ascontiguousarray(x.transpose(0, 2, 1)).astype(BF16_NP)

    in_maps = []
    for core in range(NCORES):
        vh = slice(VH * core, VH * (core + 1))
        qrows = slice(QCH * core, QCH * (core + 1))
        krows = slice(KEY + QCH * core, KEY + QCH * (core + 1))
        vrows = slice(2 * KEY + VCH * core, 2 * KEY + VCH * (core + 1))
        zrows = slice(ZCH * core, ZCH * (core + 1))

        Wt = np.concatenate(
            [W_qkv[qrows], W_qkv[krows], W_qkv[vrows], W_z[zrows],
             W_b[vh], np.zeros((32 - VH, C), np.float32),
             W_a[vh]], axis=0)                    # [TOTCH, C]
        wt_host = np.ascontiguousarray(Wt.T).astype(BF16_NP)      # [C, TOTCH]
        wout_host = np.ascontiguousarray(
            W_out[:, VCH * core:VCH * (core + 1)].T).astype(BF16_NP)

        cw = np.concatenate([conv_w[qrows], conv_w[krows], conv_w[vrows]], 0)
        convw_host = np.ascontiguousarray(
            cw.reshape(CONVCH // 128, 128, KCONV).transpose(1, 0, 2))

        cs = np.concatenate([conv_state[:, qrows], conv_state[:, krows],
                             conv_state[:, vrows]], axis=1)       # [B,1024,4]
        halo_host = np.ascontiguousarray(
            cs[:, :, 1:4].reshape(B, CONVCH // 128, 128, 3)
            .transpose(0, 2, 1, 3)).astype(BF16_NP)

        s0_host = np.ascontiguousarray(
            rec_state[:, vh].transpose(0, 2, 1, 3)).astype(BF16_NP)
        dtb_host = np.ascontiguousarray(dt_bias[vh][:, None])
        nega_host = np.ascontiguousarray(-np.exp(A_log[vh])[:, None])
        normw_host = np.ascontiguousarray(norm_w[:, None])

        in_maps.append({
            "xt": xt_host, "wt": wt_host, "wout": wout_host,
            "convw": convw_host, "halo": halo_host, "s0": s0_host,
            "dtb": dtb_host, "nega": nega_host, "normw": normw_host,
        })

    if _NC_CACHE is None:
        _NC_CACHE = build_kernel()
    res = run_bass_kernel_spmd(_NC_CACHE, in_maps, core_ids=list(range(NCORES)))
    LAST_RESULT = res

    total = np.zeros((B, T, C), np.float32)
    for r in res.results:
        total += r["out"].astype(np.float32)
    return total
